# revision 15
# baseline (speedup 1.0000x reference)
"""Causal self-attention (B=2, T=4096, C=768, H=12, D=64) on 8 trn2 cores.

Sharding: batch*heads across cores. Core c handles batch c//4 and heads
3*(c%4) .. 3*(c%4)+2. Each core computes the QKV projection for its head
slice, full causal attention for those heads, and a partial output
projection (its heads' rows of w_out). The host sums the 4 bf16 partials
per batch in fp32 and adds b_out.

All matmul operands are bf16 (fp32 PSUM accumulation; worst-case output
rel err ~4e-3). On-core layouts:
  xT      [C, T]    input, pre-transposed on host
  tA      [128, T]  = [qT_h0 | qT_h1]   (rows 0:64 | 64:128)
  tB      [128, T]  = [kT_h0 | kT_h1]
  tD/tE   rows 64:128 = qT_h2 / kT_h2 (q2 arrives via a small sbuf->sbuf
          DMA bounce since engines cannot cross partitions)
  vaug    [128, NKT, 256] bf16 v with a ones column per head at col
          h*65+64, so the PV matmul's psum row 64 = sum(exp) = softmax
          denominators for free
  scores  [128, 1024] psum (2 banks): two k-tiles per group so one ACT
          exp instruction covers 1024 columns (the ~350-cycle ACT
          instruction overhead amortizes); diagonal-supertile scores are
          computed full-width - the above-diagonal garbage (even NaN
          from stale psum) is *replaced* by affine_select's fill
  attnOAB [128, T] = [attn_h0 | attn_h1] packed (h1 via sbuf bounce) so
          the output projection contracts K=128 in one matmul + a K=64
          matmul for h2 - matmul INSTRUCTION count is what matters on
          HW (~345 ns each vs the ~215 ns cost-model stream time)
  h0/h1 score matmuls are emitted interleaved per k-tile so consecutive
  K=64 matmuls hit alternating PE row-groups (same-row-group
  back-to-back K=64 matmuls serialize their weight loads, ~2x cost)

Softmax denominators: PV psum row 64 -> prompt DVE copy to sbuf (frees
the psum bank for the next head) -> DVE reciprocal -> one sbuf->sbuf DMA
to partition 0 (partition_broadcast ucode reads physical partition 0
only; core 0 of the Q7 cluster does the read) -> gpsimd
partition_broadcast -> DVE normalize-multiply. No DRAM round trips.

Software pipelining by emission order: attention(qb) interleaves the
closures of outproj(qb-1) and projection(qb+1) between its groups, so
the in-order engine sequencers never head-of-line block a phase
boundary (this was worth ~80 us over phase-sequential emission).
"""

import numpy as np
from ml_dtypes import bfloat16

import concourse.bass as bass
import concourse.mybir as mybir
import concourse.tile as tile
from concourse import bacc
from concourse.bass_utils import run_bass_kernel_spmd

B, T, C = 2, 4096, 768
NH, D = 12, 64
HPC = 3  # heads per core
NCORES = 8
P = 128
QB = 512           # q block == projection chunk
NQB = T // QB      # 8
NKT = T // P       # 32 k tiles
GK = 2             # k-tiles per exp group (1 or 2)
FULL_SCORES = False  # diag score MMs write only cols >= co (stale cols masked)
F32 = mybir.dt.float32
F32R = mybir.dt.float32r
BF16 = mybir.dt.bfloat16

_CACHE = {}


def _declare(nc):
    xT = nc.dram_tensor("xT", [C, T], BF16, kind="ExternalInput")
    wqk = nc.dram_tensor("wqk", [C, 3 * P], BF16, kind="ExternalInput")
    wv = nc.dram_tensor("wv", [C, 256], BF16, kind="ExternalInput")
    wo = nc.dram_tensor("wo", [HPC * D, C], BF16, kind="ExternalInput")
    out = nc.dram_tensor("out", [T, C], BF16, kind="ExternalOutput")
    return dict(xT=xT.ap(), wqk=wqk.ap(), wv=wv.ap(), wo=wo.ap(), out=out.ap())


def _build_nc():
    nc = bacc.Bacc(
        "TRN2",
        target_bir_lowering=False,
        debug=False,
        enable_asserts=False,
        num_devices=NCORES,
    )
    aps = _declare(nc)
    with tile.TileContext(nc) as tc:
        _emit(tc, nc, **aps)
    nc.compile()
    return nc


def _emit(tc, nc, xT, wqk, wv, wo, out):
    import contextlib

    ctx = contextlib.ExitStack()
    with ctx:
        # ---- persistent sbuf ----
        persist = ctx.enter_context(tc.tile_pool(name="persist", bufs=1))
        tA = persist.tile([P, T], BF16, tag="tA")
        tB = persist.tile([P, T], BF16, tag="tB")
        tD = persist.tile([P, T], BF16, tag="tD")   # 64:128 = qT_h2, 0:64 = kT_h2
        tE = persist.tile([P, T], BF16, tag="tE")   # rows 64:128 = kT_h2
        tQ2 = persist.tile([D, T], BF16, tag="tQ2")  # qT_h2 at partitions 0:64
        vaug = persist.tile([P, NKT, 256], BF16, tag="vaug")
        attnOAB = persist.tile([P, T], BF16, tag="attnOAB")
        attnO2 = persist.tile([D, T], BF16, tag="attnO2")
        attnO2B = persist.tile([P, T], BF16, tag="attnO2B")  # rows 64:128 dup
        wqk_sb = persist.tile([P, 6, 3 * P], BF16, tag="wqk")
        wv_sb = persist.tile([P, 6, 256], BF16, tag="wv")
        woAB_sb = persist.tile([P, C], BF16, tag="woAB")
        wo2_sb = persist.tile([P, C], BF16, tag="wo2")  # wo2 in BOTH halves
        ones_bf = persist.tile([P, 8], BF16, tag="ones")

        nc.sync.dma_start(out=wqk_sb[:], in_=wqk.rearrange("(co p) n -> p co n", p=P))
        nc.sync.dma_start(out=wv_sb[:], in_=wv.rearrange("(co p) n -> p co n", p=P))
        nc.sync.dma_start(out=woAB_sb[:], in_=wo[0:P, :])
        nc.sync.dma_start(out=wo2_sb[0:D, :], in_=wo[P : P + D, :])
        nc.sync.dma_start(out=wo2_sb[D:P, :], in_=wo[P : P + D, :])
        nc.gpsimd.memset(ones_bf[:], 1.0)

        def qT(h, base=0):
            # h0 rows 0:64 of tA, h1 rows 64:128 of tA;
            # h2 has copies at both bases: 0 -> tQ2 (rows 0:64), 64 -> tD hi
            if h == 2:
                return tQ2[:] if base == 0 else tD[D:P]
            return (tA[0:D], tA[D:P])[h]

        def kT(h, base=0):
            if h == 2:
                return tD[0:D] if base == 0 else tE[D:P]
            return (tB[0:D], tB[D:P])[h]

        # psum budget (8 banks): p1 2 + scores 2*2 + outT 2
        with (
            tc.tile_pool(name="xchunks", bufs=2) as xpool,
            tc.tile_pool(name="p1psum", bufs=2, space="PSUM") as p1psum,
            tc.tile_pool(name="spsum", bufs=4 // GK, space="PSUM") as spool,
            tc.tile_pool(name="opsum", bufs=2, space="PSUM") as opool,
            tc.tile_pool(name="exps", bufs=6) as epool,
            tc.tile_pool(name="smalls", bufs=4) as rpool,
        ):
            from collections import deque

            def proj_work(qb):
                """Closures emitting projection chunk qb (tokens qb*512..)."""
                qsl = slice(qb * QB, (qb + 1) * QB)
                st = {}

                def dma():
                    xt = xpool.tile([P, 6, QB], BF16, tag="xt")
                    nc.sync.dma_start(
                        out=xt[:],
                        in_=xT[:, qsl].rearrange("(co p) t -> p co t", p=P),
                    )
                    st["xt"] = xt

                def chain(ci):
                    def f():
                        ps = p1psum.tile(
                            [P, QB], F32, tag="p1", name=f"p1_{qb}_{ci}"
                        )
                        for c6 in range(6):
                            nc.tensor.matmul(
                                ps[:],
                                wqk_sb[:, c6, ci * P : (ci + 1) * P],
                                st["xt"][:, c6, :],
                                start=(c6 == 0),
                                stop=(c6 == 5),
                            )
                        if ci < 2:
                            nc.vector.tensor_copy(
                                out=(tA, tB)[ci][:, qsl], in_=ps[:]
                            )
                        else:
                            # [q2|k2]: q2 lands at rows 0:64 (tQ2), k2 at rows
                            # 64:128 (tE) directly; DMA bounces make the
                            # opposite-base copies so h2 score matmuls can
                            # alternate PE row groups like h0/h1 do.
                            nc.vector.tensor_copy(
                                out=tE[D:P, qsl], in_=ps[D:P, :]
                            )
                            nc.vector.tensor_copy(out=tQ2[:, qsl], in_=ps[0:D, :])
                            nc.sync.dma_start(out=tD[D:P, qsl], in_=tQ2[:, qsl])
                            nc.sync.dma_start(out=tD[0:D, qsl], in_=tE[D:P, qsl])
                    return f

                def vhalf(half):
                    def f():
                        ktv = qb * (QB // P) + half
                        ps2 = p1psum.tile(
                            [P, QB], F32, tag="p1", name=f"p1v_{qb}_{half}"
                        )
                        for c6 in range(6):
                            nc.tensor.matmul(
                                ps2[:, 0:256],
                                st["xt"][:, c6, half * P : (half + 1) * P],
                                wv_sb[:, c6, :],
                                start=(c6 == 0),
                                stop=(c6 == 5),
                            )
                        nc.vector.tensor_copy(
                            out=vaug[:, ktv, :], in_=ps2[:, 0:256]
                        )
                        if half == QB // P - 1:
                            # restore the ones columns the v copies overwrote
                            for h in range(HPC):
                                nc.vector.tensor_copy(
                                    out=vaug[:, qb * (QB // P) :
                                             (qb + 1) * (QB // P),
                                             h * (D + 1) + D],
                                    in_=ones_bf[:, 0 : QB // P],
                                )
                    return f

                return (
                    [dma]
                    + [chain(ci) for ci in range(3)]
                    + [vhalf(h) for h in range(QB // P)]
                )

            def outproj_work(qb):
                """Closures emitting the output projection of q block qb.
                The two K=128 (h0|h1) matmuls go first, then the two K=64 h2
                matmuls back-to-back at alternating PE row groups (attnO2 at
                base 0, its DMA'd dup attnO2B at base 64) so their streams
                overlap."""
                def tt_work(tt):
                    def f():
                        tsl = slice(tt * P, (tt + 1) * P)
                        so = rpool.tile([P, C], BF16, tag="p3out", bufs=2)
                        po1 = p1psum.tile([P, QB], F32, tag="p1",
                                          name=f"po1_{tt}")
                        po2 = p1psum.tile([P, QB], F32, tag="p1",
                                          name=f"po2_{tt}")
                        nc.tensor.matmul(
                            po1[:, 0:512], attnOAB[:, tsl],
                            woAB_sb[:, 0:512], start=True, stop=False,
                        )
                        nc.tensor.matmul(
                            po2[:, 0:256], attnOAB[:, tsl],
                            woAB_sb[:, 512:768], start=True, stop=False,
                        )
                        nc.tensor.matmul(
                            po1[:, 0:512], attnO2[:, tsl],
                            wo2_sb[0:D, 0:512], start=False, stop=True,
                        )
                        nc.tensor.matmul(
                            po2[:, 0:256], attnO2B[D:P, tsl],
                            wo2_sb[D:P, 512:768], start=False, stop=True,
                        )
                        nc.vector.tensor_copy(out=so[:, 0:512], in_=po1[:, 0:512])
                        nc.vector.tensor_copy(out=so[:, 512:768], in_=po2[:, 0:256])
                        nc.sync.dma_start(out=out[tsl, :], in_=so[:])
                    return f

                return [
                    tt_work(tt)
                    for tt in range(qb * (QB // P), (qb + 1) * (QB // P))
                ]

            def attn_scores(qb, g, hs):
                """Scores+exp+mask for k-tiles (GK*g..GK*g+GK-1) of heads hs.
                Consecutive score matmuls hit alternating PE row-groups
                (h0@0/h1@64 interleaved; solo h2 alternates its duplicated
                base by k-tile parity) - alternating K=64 streams run
                CONCURRENTLY on the two PE halves (~129 ns/mm vs ~450 ns
                same-half). Returns ex tiles for the (later-emitted) PV."""
                hs = list(hs)
                sps, exs = [], []
                for h in hs:
                    sps.append(spool.tile([P, GK * QB], F32, tag="sc",
                                          name=f"sp_{qb}_{h}_{g}"))
                for i in range(GK):
                    kt = GK * g + i
                    co = 0 if FULL_SCORES else max(0, P * (kt - 4 * qb))
                    for h, sp in zip(hs, sps):
                        base = (D if h == 1 else 0) if h != 2 else \
                            (D if (kt % 2) else 0)
                        nc.tensor.matmul(
                            sp[:, i * QB + co : (i + 1) * QB],
                            kT(h, base)[:, kt * P : (kt + 1) * P],
                            qT(h, base)[:, qb * QB + co : (qb + 1) * QB],
                            start=True,
                            stop=True,
                        )
                co0 = 0 if FULL_SCORES else max(0, P * (GK * g - 4 * qb))
                for sp in sps:
                    ex = epool.tile([P, GK * QB], BF16, tag="ex")
                    nc.scalar.activation(
                        out=ex[:, co0 : GK * QB],
                        in_=sp[:, co0 : GK * QB],
                        func=mybir.ActivationFunctionType.Exp,
                        scale=float(D) ** -0.5,
                    )
                    exs.append(ex)
                for i in range(GK):
                    kt = GK * g + i
                    j = kt - 4 * qb
                    if j >= 0:  # diagonal tile: causal mask (fill 0 at q<k)
                        for ex in exs:
                            nc.gpsimd.affine_select(
                                out=ex[:, i * QB : (i + 1) * QB],
                                in_=ex[:, i * QB : (i + 1) * QB],
                                compare_op=mybir.AluOpType.is_ge,
                                fill=0.0,
                                base=-P * j,
                                pattern=[[1, QB]],
                                channel_multiplier=-1,
                            )
                return exs

            def attn_pv(qb, g, hs, exs, outps):
                nkt = 4 * qb + 4
                for h, ex, outp in zip(hs, exs, outps):
                    for i in range(GK):
                        kt = GK * g + i
                        co = max(0, P * (kt - 4 * qb))
                        nc.tensor.matmul(
                            outp[:, co:],
                            vaug[:, kt, h * (D + 1) : (h + 1) * (D + 1)],
                            ex[:, i * QB + co : (i + 1) * QB],
                            start=(kt == 0),
                            stop=(kt == nkt - 1),
                        )

            def normalize(qb, hs, outps):
                """Softmax denominators for one or two heads at once: DVE
                reciprocal of psum row 64, one sbuf->sbuf DMA to move the
                rows to partition 0 (partition_broadcast's ucode reads
                physical partition 0 only), one gpsimd broadcast, then the
                normalizing multiplies. The prompt half (psum->sbuf copy +
                reciprocal) runs now so the opsum banks free; the multiplies
                wait on the DMA+broadcast roundtrip (us-scale on HW), so
                they are RETURNED as a closure and metered into the filler
                stream - emitting them inline would head-of-line block the
                in-order DVE queue and stall PE on psum frees."""
                qsl = slice(qb * QB, (qb + 1) * QB)
                nh = len(hs)
                ot = rpool.tile([D + 1, nh, QB], F32, tag=f"ot{nh}", bufs=3)
                for i, outp in enumerate(outps):
                    nc.vector.tensor_copy(out=ot[:, i, :], in_=outp[:])
                rt = rpool.tile([D + 1, nh, QB], F32, tag=f"recip{nh}", bufs=2)
                nc.vector.reciprocal(
                    out=rt[D : D + 1, :, :], in_=ot[D : D + 1, :, :]
                )
                rb = rpool.tile([1, nh, QB], F32, tag=f"rb{nh}", bufs=2)
                nc.sync.dma_start(out=rb[:], in_=rt[D : D + 1, :, :])
                rbc = rpool.tile([D, nh, QB], F32, tag=f"rbc{nh}", bufs=3)
                nc.gpsimd.partition_broadcast(rbc[:], rb[:])

                def muls():
                    for i, h in enumerate(hs):
                        if h == 0:
                            dst = attnOAB[0:D, qsl]
                        elif h == 2:
                            dst = attnO2[:, qsl]
                        else:
                            # h1 belongs at partitions 64:128 of attnOAB,
                            # which engines cannot reach from lanes 0:64 -
                            # stage and DMA-bounce (sbuf->sbuf)
                            sg = rpool.tile([D, QB], BF16, tag="sg", bufs=2,
                                            name=f"sg_{qb}")
                            dst = sg[:]
                        nc.vector.tensor_mul(
                            out=dst, in0=ot[0:D, i, :], in1=rbc[:, i, :]
                        )
                        if h == 1:
                            nc.sync.dma_start(out=attnOAB[D:P, qsl], in_=dst)
                        elif h == 2:
                            # dup at base 64 so outproj K=64 matmuls pair
                            nc.sync.dma_start(out=attnO2B[D:P, qsl], in_=dst)
                return muls

            # Software pipeline by emission order. Two mechanisms keep the
            # in-order PE sequencer dense (any PE gap also drops the PE
            # p-state, doubling subsequent matmul cost until ~3us of
            # continuous execution):
            #  - PV of group g is emitted after scores of group g+2, so PE
            #    never head-of-line blocks on exp(g)/mask(g).
            #  - filler closures (proj of qb+1, outproj of any finished qb)
            #    are METERED across groups: the h01 phases are ACT-bound
            #    (2 exps = 2.06us vs ~1.6us of PE work per group), so every
            #    group needs ~0.5us of unrelated PE work or PE idles.
            #    proj(qb+1) must drain before attention(qb+1) (deadline);
            #    outproj reads persistent sbuf and can float arbitrarily.
            for w in proj_work(0):
                w()
            pend_proj = deque(proj_work(1) if NQB > 1 else [])
            pend_out = deque()
            total_groups = sum(4 * q + 4 for q in range(NQB))
            meter = {"gleft": total_groups, "pcredit": 0.0, "ocredit": 0.0,
                     "pq_groups": 0}

            def fill():
                """Emit a metered share of pending filler closures."""
                meter["gleft"] -= 1
                meter["pq_groups"] = max(1, meter["pq_groups"] - 1)
                if pend_proj:
                    meter["pcredit"] += len(pend_proj) / meter["pq_groups"]
                    while pend_proj and meter["pcredit"] >= 1.0:
                        pend_proj.popleft()()
                        meter["pcredit"] -= 1.0
                if pend_out:
                    meter["ocredit"] += len(pend_out) / max(1, meter["gleft"])
                    while pend_out and meter["ocredit"] >= 1.0:
                        pend_out.popleft()()
                        meter["ocredit"] -= 1.0

            def attn_phase(qb, hs, outps):
                hist = deque()
                for g in range((4 * qb + 4) // GK):
                    exs = attn_scores(qb, g, hs)
                    hist.append((g, exs))
                    if len(hist) > 2:
                        pg, pexs = hist.popleft()
                        attn_pv(qb, pg, hs, pexs, outps)
                    fill()
                while hist:
                    pg, pexs = hist.popleft()
                    attn_pv(qb, pg, hs, pexs, outps)
                    fill()

            for qb in range(NQB):
                # proj(qb+1) must fully drain during attention(qb): meter it
                # over this qb's groups only
                meter["pq_groups"] = 4 * qb + 4
                outp0 = opool.tile([D + 1, QB], F32, tag="outT", name=f"o0_{qb}")
                outp1 = opool.tile([D + 1, QB], F32, tag="outT", name=f"o1_{qb}")
                attn_phase(qb, (0, 1), (outp0, outp1))
                muls01 = normalize(qb, (0, 1), (outp0, outp1))
                pend_out.append(muls01)
                outp2 = opool.tile([D + 1, QB], F32, tag="outT", name=f"o2_{qb}")
                attn_phase(qb, (2,), (outp2,))
                muls2 = normalize(qb, (2,), (outp2,))
                pend_out.append(muls2)
                while pend_proj:
                    pend_proj.popleft()()
                pend_out.extend(outproj_work(qb))
                # bound the outproj/mul float to ~one qb so WAR reuse of the
                # ot/rbc tiles never stalls the broadcast chain
                while len(pend_out) > 6:
                    pend_out.popleft()()
                if qb + 2 < NQB:
                    pend_proj.extend(proj_work(qb + 2))
            while pend_proj:
                pend_proj.popleft()()
            while pend_out:
                pend_out.popleft()()


def _get_nc():
    if "nc" not in _CACHE:
        _CACHE["nc"] = _build_nc()
    return _CACHE["nc"]


def _shard_inputs(x, w_qkv, w_out):
    """Build per-core input maps."""
    x = np.asarray(x, dtype=np.float32)
    w_qkv = np.asarray(w_qkv, dtype=np.float32)
    w_out = np.asarray(w_out, dtype=np.float32)
    xTs = [np.ascontiguousarray(x[b].T) for b in range(B)]
    in_maps = []
    for c in range(NCORES):
        b = c // 4
        heads = [HPC * (c % 4) + i for i in range(HPC)]
        q = [w_qkv[:, h * D : (h + 1) * D] for h in heads]
        k = [w_qkv[:, C + h * D : C + (h + 1) * D] for h in heads]
        wqk = np.concatenate([q[0], q[1], k[0], k[1], q[2], k[2]], axis=1)
        wv = np.zeros((C, 256), dtype=np.float32)
        for i, h in enumerate(heads):
            wv[:, i * (D + 1) : i * (D + 1) + D] = w_qkv[
                :, 2 * C + h * D : 2 * C + (h + 1) * D
            ]
        wo = np.concatenate(
            [w_out[h * D : (h + 1) * D, :] for h in heads], axis=0
        )  # [HPC*D, C]
        in_maps.append(
            {
                "xT": xTs[b].astype(bfloat16),
                "wqk": np.ascontiguousarray(wqk).astype(bfloat16),
                "wv": wv.astype(bfloat16),
                "wo": np.ascontiguousarray(wo).astype(bfloat16),
            }
        )
    return in_maps


def kernel(x, w_qkv, w_out, b_out):
    nc = _get_nc()
    in_maps = _shard_inputs(x, w_qkv, w_out)
    res = run_bass_kernel_spmd(nc, in_maps, core_ids=list(range(NCORES)))
    b_out = np.asarray(b_out, dtype=np.float32)
    outs = []
    for b in range(B):
        acc = res.results[4 * b]["out"].astype(np.float32).copy()
        for c in range(4 * b + 1, 4 * b + 4):
            acc += res.results[c]["out"]
        outs.append(acc + b_out[None, :])
    return np.stack(outs, axis=0)



# revision 17
# speedup vs baseline: 1.2693x; 1.2693x over previous
"""Causal self-attention (B=2, T=4096, C=768, H=12, D=64) on 8 trn2 cores.

Sharding: batch*heads across cores. Core c handles batch c//4 and heads
3*(c%4) .. 3*(c%4)+2. Each core computes the QKV projection for its head
slice, full causal attention for those heads, and a partial output
projection (its heads' rows of w_out). The host sums the 4 bf16 partials
per batch in fp32 and adds b_out.

All matmul operands are bf16 (fp32 PSUM accumulation; worst-case output
rel err ~4e-3). On-core layouts:
  xT      [C, T]    input, pre-transposed on host
  tA      [128, T]  = [qT_h0 | qT_h1]   (rows 0:64 | 64:128)
  tB      [128, T]  = [kT_h0 | kT_h1]
  tD/tE   rows 64:128 = qT_h2 / kT_h2 (q2 arrives via a small sbuf->sbuf
          DMA bounce since engines cannot cross partitions)
  vaug    [128, NKT, 256] bf16 v with a ones column per head at col
          h*65+64, so the PV matmul's psum row 64 = sum(exp) = softmax
          denominators for free
  scores  [128, 1024] psum (2 banks): two k-tiles per group so one ACT
          exp instruction covers 1024 columns (the ~350-cycle ACT
          instruction overhead amortizes); diagonal-supertile scores are
          computed full-width - the above-diagonal garbage (even NaN
          from stale psum) is *replaced* by affine_select's fill
  attnOAB [128, T] = [attn_h0 | attn_h1] packed (h1 via sbuf bounce) so
          the output projection contracts K=128 in one matmul + a K=64
          matmul for h2 - matmul INSTRUCTION count is what matters on
          HW (~345 ns each vs the ~215 ns cost-model stream time)
  h0/h1 score matmuls are emitted interleaved per k-tile so consecutive
  K=64 matmuls hit alternating PE row-groups (same-row-group
  back-to-back K=64 matmuls serialize their weight loads, ~2x cost)

Softmax denominators: PV psum row 64 -> prompt DVE copy to sbuf (frees
the psum bank for the next head) -> DVE reciprocal -> one sbuf->sbuf DMA
to partition 0 (partition_broadcast ucode reads physical partition 0
only; core 0 of the Q7 cluster does the read) -> gpsimd
partition_broadcast -> DVE normalize-multiply. No DRAM round trips.

Software pipelining by emission order: attention(qb) interleaves the
closures of outproj(qb-1) and projection(qb+1) between its groups, so
the in-order engine sequencers never head-of-line block a phase
boundary (this was worth ~80 us over phase-sequential emission).
"""

import numpy as np
from ml_dtypes import bfloat16

import concourse.bass as bass
import concourse.mybir as mybir
import concourse.tile as tile
from concourse import bacc
from concourse.bass_utils import run_bass_kernel_spmd

B, T, C = 2, 4096, 768
NH, D = 12, 64
HPC = 3  # heads per core
NCORES = 8
P = 128
QB = 512           # q block == projection chunk
NQB = T // QB      # 8
NKT = T // P       # 32 k tiles
GK = 2             # k-tiles per exp group (1 or 2)
FULL_SCORES = False  # diag score MMs write only cols >= co (stale cols masked)
F32 = mybir.dt.float32
F32R = mybir.dt.float32r
BF16 = mybir.dt.bfloat16

_CACHE = {}


def _declare(nc):
    xT = nc.dram_tensor("xT", [C, T], BF16, kind="ExternalInput")
    wqk = nc.dram_tensor("wqk", [C, 3 * P], BF16, kind="ExternalInput")
    wv = nc.dram_tensor("wv", [C, 256], BF16, kind="ExternalInput")
    wo = nc.dram_tensor("wo", [HPC * D, C], BF16, kind="ExternalInput")
    out = nc.dram_tensor("out", [T, C], BF16, kind="ExternalOutput")
    return dict(xT=xT.ap(), wqk=wqk.ap(), wv=wv.ap(), wo=wo.ap(), out=out.ap())


def _build_nc():
    nc = bacc.Bacc(
        "TRN2",
        target_bir_lowering=False,
        debug=False,
        enable_asserts=False,
        num_devices=NCORES,
    )
    aps = _declare(nc)
    with tile.TileContext(nc) as tc:
        _emit(tc, nc, **aps)
    nc.compile()
    return nc


def _emit(tc, nc, xT, wqk, wv, wo, out):
    import contextlib

    ctx = contextlib.ExitStack()
    with ctx:
        # ---- persistent sbuf ----
        persist = ctx.enter_context(tc.tile_pool(name="persist", bufs=1))
        tA = persist.tile([P, T], BF16, tag="tA")
        tB = persist.tile([P, T], BF16, tag="tB")
        tD = persist.tile([P, T], BF16, tag="tD")   # 64:128 = qT_h2, 0:64 = kT_h2
        tE = persist.tile([P, T], BF16, tag="tE")   # rows 64:128 = kT_h2
        tQ2 = persist.tile([D, T], BF16, tag="tQ2")  # qT_h2 at partitions 0:64
        vaug = persist.tile([P, NKT, 256], BF16, tag="vaug")
        attnOAB = persist.tile([P, T], BF16, tag="attnOAB")
        attnO2 = persist.tile([D, T], BF16, tag="attnO2")
        attnO2B = persist.tile([P, T], BF16, tag="attnO2B")  # rows 64:128 dup
        wqk_sb = persist.tile([P, 6, 3 * P], BF16, tag="wqk")
        wv_sb = persist.tile([P, 6, 256], BF16, tag="wv")
        woAB_sb = persist.tile([P, C], BF16, tag="woAB")
        wo2_sb = persist.tile([P, C], BF16, tag="wo2")  # wo2 in BOTH halves
        ones_bf = persist.tile([P, 8], BF16, tag="ones")

        nc.sync.dma_start(out=wqk_sb[:], in_=wqk.rearrange("(co p) n -> p co n", p=P))
        nc.sync.dma_start(out=wv_sb[:], in_=wv.rearrange("(co p) n -> p co n", p=P))
        nc.sync.dma_start(out=woAB_sb[:], in_=wo[0:P, :])
        nc.sync.dma_start(out=wo2_sb[0:D, :], in_=wo[P : P + D, :])
        nc.sync.dma_start(out=wo2_sb[D:P, :], in_=wo[P : P + D, :])
        nc.gpsimd.memset(ones_bf[:], 1.0)

        def qT(h, base=0):
            # h0 rows 0:64 of tA, h1 rows 64:128 of tA;
            # h2 has copies at both bases: 0 -> tQ2 (rows 0:64), 64 -> tD hi
            if h == 2:
                return tQ2[:] if base == 0 else tD[D:P]
            return (tA[0:D], tA[D:P])[h]

        def kT(h, base=0):
            if h == 2:
                return tD[0:D] if base == 0 else tE[D:P]
            return (tB[0:D], tB[D:P])[h]

        # psum budget (8 banks): p1 2 + scores 2*2 + outT 2
        with (
            tc.tile_pool(name="xchunks", bufs=2) as xpool,
            tc.tile_pool(name="p1psum", bufs=2, space="PSUM") as p1psum,
            tc.tile_pool(name="spsum", bufs=4 // GK, space="PSUM") as spool,
            tc.tile_pool(name="opsum", bufs=2, space="PSUM") as opool,
            tc.tile_pool(name="exps", bufs=6) as epool,
            tc.tile_pool(name="smalls", bufs=4) as rpool,
        ):
            from collections import deque

            def proj_work(qb):
                """Closures emitting projection chunk qb (tokens qb*512..)."""
                qsl = slice(qb * QB, (qb + 1) * QB)
                st = {}

                def dma():
                    xt = xpool.tile([P, 6, QB], BF16, tag="xt")
                    nc.sync.dma_start(
                        out=xt[:],
                        in_=xT[:, qsl].rearrange("(co p) t -> p co t", p=P),
                    )
                    st["xt"] = xt

                def chain(ci):
                    def f():
                        ps = p1psum.tile(
                            [P, QB], F32, tag="p1", name=f"p1_{qb}_{ci}"
                        )
                        for c6 in range(6):
                            nc.tensor.matmul(
                                ps[:],
                                wqk_sb[:, c6, ci * P : (ci + 1) * P],
                                st["xt"][:, c6, :],
                                start=(c6 == 0),
                                stop=(c6 == 5),
                            )
                        if ci < 2:
                            nc.vector.tensor_copy(
                                out=(tA, tB)[ci][:, qsl], in_=ps[:]
                            )
                        else:
                            # [q2|k2]: q2 lands at rows 0:64 (tQ2), k2 at rows
                            # 64:128 (tE) directly; DMA bounces make the
                            # opposite-base copies so h2 score matmuls can
                            # alternate PE row groups like h0/h1 do.
                            nc.vector.tensor_copy(
                                out=tE[D:P, qsl], in_=ps[D:P, :]
                            )
                            nc.vector.tensor_copy(out=tQ2[:, qsl], in_=ps[0:D, :])
                            nc.sync.dma_start(out=tD[D:P, qsl], in_=tQ2[:, qsl])
                            nc.sync.dma_start(out=tD[0:D, qsl], in_=tE[D:P, qsl])
                    return f

                def vhalf(half):
                    def f():
                        ktv = qb * (QB // P) + half
                        ps2 = p1psum.tile(
                            [P, QB], F32, tag="p1", name=f"p1v_{qb}_{half}"
                        )
                        for c6 in range(6):
                            nc.tensor.matmul(
                                ps2[:, 0:256],
                                st["xt"][:, c6, half * P : (half + 1) * P],
                                wv_sb[:, c6, :],
                                start=(c6 == 0),
                                stop=(c6 == 5),
                            )
                        nc.vector.tensor_copy(
                            out=vaug[:, ktv, :], in_=ps2[:, 0:256]
                        )
                        if half == QB // P - 1:
                            # restore the ones columns the v copies overwrote
                            for h in range(HPC):
                                nc.vector.tensor_copy(
                                    out=vaug[:, qb * (QB // P) :
                                             (qb + 1) * (QB // P),
                                             h * (D + 1) + D],
                                    in_=ones_bf[:, 0 : QB // P],
                                )
                    return f

                return (
                    [dma]
                    + [chain(ci) for ci in range(3)]
                    + [vhalf(h) for h in range(QB // P)]
                )

            def outproj_work(qb):
                """Closures emitting the output projection of q block qb.
                The two K=128 (h0|h1) matmuls go first, then the two K=64 h2
                matmuls back-to-back at alternating PE row groups (attnO2 at
                base 0, its DMA'd dup attnO2B at base 64) so their streams
                overlap."""
                def tt_work(tt):
                    def f():
                        tsl = slice(tt * P, (tt + 1) * P)
                        so = rpool.tile([P, C], BF16, tag="p3out", bufs=2)
                        po1 = p1psum.tile([P, QB], F32, tag="p1",
                                          name=f"po1_{tt}")
                        po2 = p1psum.tile([P, QB], F32, tag="p1",
                                          name=f"po2_{tt}")
                        nc.tensor.matmul(
                            po1[:, 0:512], attnOAB[:, tsl],
                            woAB_sb[:, 0:512], start=True, stop=False,
                        )
                        nc.tensor.matmul(
                            po2[:, 0:256], attnOAB[:, tsl],
                            woAB_sb[:, 512:768], start=True, stop=False,
                        )
                        nc.tensor.matmul(
                            po1[:, 0:512], attnO2[:, tsl],
                            wo2_sb[0:D, 0:512], start=False, stop=True,
                        )
                        nc.tensor.matmul(
                            po2[:, 0:256], attnO2B[D:P, tsl],
                            wo2_sb[D:P, 512:768], start=False, stop=True,
                        )
                        nc.vector.tensor_copy(out=so[:, 0:512], in_=po1[:, 0:512])
                        nc.vector.tensor_copy(out=so[:, 512:768], in_=po2[:, 0:256])
                        nc.sync.dma_start(out=out[tsl, :], in_=so[:])
                    return f

                return [
                    tt_work(tt)
                    for tt in range(qb * (QB // P), (qb + 1) * (QB // P))
                ]

            def attn_scores(qb, g, hs):
                """Scores+exp+mask for k-tiles (GK*g..GK*g+GK-1) of heads hs.
                Consecutive score matmuls hit alternating PE row-groups
                (h0@0/h1@64 interleaved; solo h2 alternates its duplicated
                base by k-tile parity) - alternating K=64 streams run
                CONCURRENTLY on the two PE halves (~129 ns/mm vs ~450 ns
                same-half). Returns ex tiles for the (later-emitted) PV."""
                hs = list(hs)
                sps, exs = [], []
                for h in hs:
                    sps.append(spool.tile([P, GK * QB], F32, tag="sc",
                                          name=f"sp_{qb}_{h}_{g}"))
                for i in range(GK):
                    kt = GK * g + i
                    co = 0 if FULL_SCORES else max(0, P * (kt - 4 * qb))
                    for h, sp in zip(hs, sps):
                        base = (D if h == 1 else 0) if h != 2 else \
                            (D if (kt % 2) else 0)
                        nc.tensor.matmul(
                            sp[:, i * QB + co : (i + 1) * QB],
                            kT(h, base)[:, kt * P : (kt + 1) * P],
                            qT(h, base)[:, qb * QB + co : (qb + 1) * QB],
                            start=True,
                            stop=True,
                        )
                co0 = 0 if FULL_SCORES else max(0, P * (GK * g - 4 * qb))
                import os as _os
                if _os.environ.get("KPROBE") == "exp8":
                    co0 = GK * QB - 128
                for sp in sps:
                    ex = epool.tile([P, GK * QB], BF16, tag="ex")
                    nc.scalar.activation(
                        out=ex[:, co0 : GK * QB],
                        in_=sp[:, co0 : GK * QB],
                        func=mybir.ActivationFunctionType.Exp,
                        scale=float(D) ** -0.5,
                    )
                    exs.append(ex)
                for i in range(GK):
                    kt = GK * g + i
                    j = kt - 4 * qb
                    if j >= 0:  # diagonal tile: causal mask (fill 0 at q<k)
                        for ex in exs:
                            nc.gpsimd.affine_select(
                                out=ex[:, i * QB : (i + 1) * QB],
                                in_=ex[:, i * QB : (i + 1) * QB],
                                compare_op=mybir.AluOpType.is_ge,
                                fill=0.0,
                                base=-P * j,
                                pattern=[[1, QB]],
                                channel_multiplier=-1,
                            )
                return exs

            def attn_pv(qb, g, hs, exs, outps):
                nkt = 4 * qb + 4
                for h, ex, outp in zip(hs, exs, outps):
                    for i in range(GK):
                        kt = GK * g + i
                        co = max(0, P * (kt - 4 * qb))
                        nc.tensor.matmul(
                            outp[:, co:],
                            vaug[:, kt, h * (D + 1) : (h + 1) * (D + 1)],
                            ex[:, i * QB + co : (i + 1) * QB],
                            start=(kt == 0),
                            stop=(kt == nkt - 1),
                        )

            def normalize(qb, hs, outps):
                """Softmax denominators for one or two heads at once: DVE
                reciprocal of psum row 64, one sbuf->sbuf DMA to move the
                rows to partition 0 (partition_broadcast's ucode reads
                physical partition 0 only), one gpsimd broadcast, then the
                normalizing multiplies. The prompt half (psum->sbuf copy +
                reciprocal) runs now so the opsum banks free; the multiplies
                wait on the DMA+broadcast roundtrip (us-scale on HW), so
                they are RETURNED as a closure and metered into the filler
                stream - emitting them inline would head-of-line block the
                in-order DVE queue and stall PE on psum frees."""
                qsl = slice(qb * QB, (qb + 1) * QB)
                nh = len(hs)
                ot = rpool.tile([D + 1, nh, QB], F32, tag=f"ot{nh}", bufs=3)
                for i, outp in enumerate(outps):
                    nc.vector.tensor_copy(out=ot[:, i, :], in_=outp[:])
                rt = rpool.tile([D + 1, nh, QB], F32, tag=f"recip{nh}", bufs=2)
                nc.vector.reciprocal(
                    out=rt[D : D + 1, :, :], in_=ot[D : D + 1, :, :]
                )
                rb = rpool.tile([1, nh, QB], F32, tag=f"rb{nh}", bufs=2)
                nc.sync.dma_start(out=rb[:], in_=rt[D : D + 1, :, :])
                rbc = rpool.tile([D, nh, QB], F32, tag=f"rbc{nh}", bufs=3)
                nc.gpsimd.partition_broadcast(rbc[:], rb[:])

                def muls():
                    for i, h in enumerate(hs):
                        if h == 0:
                            dst = attnOAB[0:D, qsl]
                        elif h == 2:
                            dst = attnO2[:, qsl]
                        else:
                            # h1 belongs at partitions 64:128 of attnOAB,
                            # which engines cannot reach from lanes 0:64 -
                            # stage and DMA-bounce (sbuf->sbuf)
                            sg = rpool.tile([D, QB], BF16, tag="sg", bufs=2,
                                            name=f"sg_{qb}")
                            dst = sg[:]
                        nc.vector.tensor_mul(
                            out=dst, in0=ot[0:D, i, :], in1=rbc[:, i, :]
                        )
                        if h == 1:
                            nc.sync.dma_start(out=attnOAB[D:P, qsl], in_=dst)
                        elif h == 2:
                            # dup at base 64 so outproj K=64 matmuls pair
                            nc.sync.dma_start(out=attnO2B[D:P, qsl], in_=dst)
                return muls

            # Software pipeline by emission order. Two mechanisms keep the
            # in-order PE sequencer dense (any PE gap also drops the PE
            # p-state, doubling subsequent matmul cost until ~3us of
            # continuous execution):
            #  - PV of group g is emitted after scores of group g+2, so PE
            #    never head-of-line blocks on exp(g)/mask(g).
            #  - filler closures (proj of qb+1, outproj of any finished qb)
            #    are METERED across groups: the h01 phases are ACT-bound
            #    (2 exps = 2.06us vs ~1.6us of PE work per group), so every
            #    group needs ~0.5us of unrelated PE work or PE idles.
            #    proj(qb+1) must drain before attention(qb+1) (deadline);
            #    outproj reads persistent sbuf and can float arbitrarily.
            for w in proj_work(0):
                w()
            pend_proj = deque(proj_work(1) if NQB > 1 else [])
            pend_out = deque()
            total_groups = sum(4 * q + 4 for q in range(NQB))
            meter = {"gleft": total_groups, "pcredit": 0.0, "ocredit": 0.0,
                     "pq_groups": 0}

            def fill():
                """Emit a metered share of pending filler closures."""
                meter["gleft"] -= 1
                meter["pq_groups"] = max(1, meter["pq_groups"] - 1)
                if pend_proj:
                    meter["pcredit"] += len(pend_proj) / meter["pq_groups"]
                    while pend_proj and meter["pcredit"] >= 1.0:
                        pend_proj.popleft()()
                        meter["pcredit"] -= 1.0
                if pend_out:
                    meter["ocredit"] += len(pend_out) / max(1, meter["gleft"])
                    while pend_out and meter["ocredit"] >= 1.0:
                        pend_out.popleft()()
                        meter["ocredit"] -= 1.0

            def attn_phase(qb, hs, outps):
                hist = deque()
                for g in range((4 * qb + 4) // GK):
                    exs = attn_scores(qb, g, hs)
                    hist.append((g, exs))
                    if len(hist) > 2:
                        pg, pexs = hist.popleft()
                        attn_pv(qb, pg, hs, pexs, outps)
                    fill()
                while hist:
                    pg, pexs = hist.popleft()
                    attn_pv(qb, pg, hs, pexs, outps)
                    fill()

            for qb in range(NQB):
                # proj(qb+1) must fully drain during attention(qb): meter it
                # over this qb's groups only
                meter["pq_groups"] = 4 * qb + 4
                outp0 = opool.tile([D + 1, QB], F32, tag="outT", name=f"o0_{qb}")
                outp1 = opool.tile([D + 1, QB], F32, tag="outT", name=f"o1_{qb}")
                attn_phase(qb, (0, 1), (outp0, outp1))
                normalize(qb, (0, 1), (outp0, outp1))()
                outp2 = opool.tile([D + 1, QB], F32, tag="outT", name=f"o2_{qb}")
                attn_phase(qb, (2,), (outp2,))
                normalize(qb, (2,), (outp2,))()
                while pend_proj:
                    pend_proj.popleft()()
                pend_out.extend(outproj_work(qb))
                if qb + 2 < NQB:
                    pend_proj.extend(proj_work(qb + 2))
            while pend_proj:
                pend_proj.popleft()()
            while pend_out:
                pend_out.popleft()()


def _get_nc():
    if "nc" not in _CACHE:
        _CACHE["nc"] = _build_nc()
    return _CACHE["nc"]


def _shard_inputs(x, w_qkv, w_out):
    """Build per-core input maps."""
    x = np.asarray(x, dtype=np.float32)
    w_qkv = np.asarray(w_qkv, dtype=np.float32)
    w_out = np.asarray(w_out, dtype=np.float32)
    xTs = [np.ascontiguousarray(x[b].T) for b in range(B)]
    in_maps = []
    for c in range(NCORES):
        b = c // 4
        heads = [HPC * (c % 4) + i for i in range(HPC)]
        q = [w_qkv[:, h * D : (h + 1) * D] for h in heads]
        k = [w_qkv[:, C + h * D : C + (h + 1) * D] for h in heads]
        wqk = np.concatenate([q[0], q[1], k[0], k[1], q[2], k[2]], axis=1)
        wv = np.zeros((C, 256), dtype=np.float32)
        for i, h in enumerate(heads):
            wv[:, i * (D + 1) : i * (D + 1) + D] = w_qkv[
                :, 2 * C + h * D : 2 * C + (h + 1) * D
            ]
        wo = np.concatenate(
            [w_out[h * D : (h + 1) * D, :] for h in heads], axis=0
        )  # [HPC*D, C]
        in_maps.append(
            {
                "xT": xTs[b].astype(bfloat16),
                "wqk": np.ascontiguousarray(wqk).astype(bfloat16),
                "wv": wv.astype(bfloat16),
                "wo": np.ascontiguousarray(wo).astype(bfloat16),
            }
        )
    return in_maps


def kernel(x, w_qkv, w_out, b_out):
    nc = _get_nc()
    in_maps = _shard_inputs(x, w_qkv, w_out)
    res = run_bass_kernel_spmd(nc, in_maps, core_ids=list(range(NCORES)))
    b_out = np.asarray(b_out, dtype=np.float32)
    outs = []
    for b in range(B):
        acc = res.results[4 * b]["out"].astype(np.float32).copy()
        for c in range(4 * b + 1, 4 * b + 4):
            acc += res.results[c]["out"]
        outs.append(acc + b_out[None, :])
    return np.stack(outs, axis=0)

